# revision 1
# baseline (speedup 1.0000x reference)
"""Trainium2 Bass kernel: collaborative-filtering score (segment_reduce problem).

Math (per batch element b):
    ubf[u]    = masked mean over nonzero entries of rating_mtx[u, :]
    score[b]  = sum_u  S[user_b, u] * (R[u, item_b] - ubf[u])
    out[b]    = 5 * sigmoid(score[b] + user_bias[user_b] + item_bias[item_b] + gb)

Distribution: the u axis (8192 users) is split across 8 NeuronCores.
Core k holds the column slice S[:, k*1024:(k+1)*1024] and the row slice
R[k*1024:(k+1)*1024, :] stored TRANSPOSED ([items, users_local]). Both
per-batch operands arrive via transposed row-gathers (dma_gather
transpose=True), landing as [u'-partition x batch-free] tiles:
    G_T[u', b] = S_slice[user_b, u']      (by user index)
    A_T[u', b] = R_sliceT[item_b, u'] - 2.5   (pre-centered, exact in bf16)
Each core computes ubf for its own 1024 users on-chip (PE column sums of
values and of the nonzero mask). In the transposed layout ubf is a
per-partition scalar, so scalar_tensor_tensor fuses (A - ubf)*G into ONE
DVE op per f-group (the subtract happens in fp32 internally), and the
u'-reduction runs on the idle Tensor engine as ones-weighted M=1 matmuls
accumulating in fp32 PSUM. Scores come out batch-major, so no final
transpose is needed. Bias terms ride as hi/lo-split augmented gather
columns on core 0 only, folding into the same dot product. The 8 per-core
partial score vectors are AllReduced; every core applies 5*sigmoid and
writes the full [8192] output (core 0's is returned).

Tables are bf16 (ratings-2.5 and biases exact via hi/lo); measured error vs
the fp32 reference: 2.0e-2 max abs on a 0..5 output (0.4% of scale).
Measured HW exec: 253 us/core (vs 457 us for the fp32 variant of this
kernel, layout_t=False + use_bf16=False).
"""

import sys
from dataclasses import dataclass

import numpy as np

if "/opt/trn_rl_repo" not in sys.path:
    sys.path.insert(0, "/opt/trn_rl_repo")


@dataclass(frozen=True)
class Cfg:
    n_users: int = 8192
    n_items: int = 4096
    batch: int = 8192
    n_cores: int = 8
    chunk: int = 512  # gather indices per dma_gather instruction
    use_bf16: bool = True
    layout_t: bool = True  # transposed gathers + PE reduce (bf16 only)
    g_hilo: bool = False  # hi/lo split sim table (fp32-quality products)

    @property
    def ul(self) -> int:  # users per core
        return self.n_users // self.n_cores

    @property
    def w(self) -> int:
        # gather-row width: UL data cols + 4 bias cols, padded so the row
        # byte count is a multiple of 256 (dma_gather constraint)
        align = 128 if self.use_bf16 else 64
        return ((self.ul + 4 + align - 1) // align) * align


def build_program(cfg: Cfg):
    if cfg.layout_t:
        return build_program_t(cfg)
    return build_program_b(cfg)


def build_program_t(cfg: Cfg):
    """Layout-T: transposed gathers put tiles in [u'-partition, batch-free]
    form. DVE only does the elementwise products (bf16, 2x packed); the
    partition-axis reduction over u' runs on the Tensor engine as M=1/M=2
    matmuls with `ones` weights, and the ubf mean-correction term
    sum_u G[b,u]*ubf[u] is folded into the SAME PE pass using [ubf_hi,
    ubf_lo] as a 2-column weight matrix (exact fp32-quality ubf). Scores
    come out batch-major on partition 0, so no final transpose is needed.
    """
    from concourse import bacc, mybir, tile

    f32 = mybir.dt.float32
    i16 = mybir.dt.int16
    bf16 = mybir.dt.bfloat16
    Alu = mybir.AluOpType
    Act = mybir.ActivationFunctionType
    assert cfg.use_bf16

    U, I, B, W, UL = cfg.n_users, cfg.n_items, cfg.batch, cfg.w, cfg.ul
    F = W // 128  # f-groups per gather row
    GF = 2 * F if cfg.g_hilo else F  # f-groups in the sim table
    WG = 128 * GF
    CH = cfg.chunk
    NCH = B // CH
    ICH = I // 128
    IDXC = B // 16
    BC = B // 128  # columns per partition in the final [128, BC] view
    groups = [list(range(cfg.n_cores))]
    nslices = [(o, min(512, UL - o)) for o in range(0, UL, 512)]

    nc = bacc.Bacc(
        None, target_bir_lowering=False, debug=False, num_swdge_queues=2
    )

    sim_t = nc.dram_tensor("sim_aug", [U, WG], bf16, kind="ExternalInput")
    rtt_t = nc.dram_tensor("ratt_aug", [I, W], bf16, kind="ExternalInput")
    uidx_t = nc.dram_tensor("uidx", [128, IDXC], i16, kind="ExternalInput")
    iidx_t = nc.dram_tensor("iidx", [128, IDXC], i16, kind="ExternalInput")
    out_t = nc.dram_tensor("out", [B], f32, kind="ExternalOutput")

    with tile.TileContext(nc) as tc:
        with (
            tc.tile_pool(name="static", bufs=1) as st,
            tc.tile_pool(name="rstream", bufs=4) as rpool,
            tc.tile_pool(name="maskp", bufs=2) as mpool,
            tc.tile_pool(name="gpool", bufs=3) as gpool,
            tc.tile_pool(name="apool", bufs=3) as apool,
            tc.tile_pool(name="prodp", bufs=2) as ppool,
            tc.tile_pool(name="psA", bufs=1, space="PSUM") as psA,
            tc.tile_pool(name="psB", bufs=2, space="PSUM") as psB,
            tc.tile_pool(name="dram", bufs=1, space="DRAM") as dram,
        ):
            # ---- static setup ----
            ones_w = st.tile([128, 1], bf16)
            nc.gpsimd.memset(ones_w[:], 1.0)
            uidx_sb = st.tile([128, IDXC], i16)
            nc.sync.dma_start(out=uidx_sb[:], in_=uidx_t[:])
            iidx_sb = st.tile([128, IDXC], i16)
            nc.sync.dma_start(out=iidx_sb[:], in_=iidx_t[:])

            # ---- ubf: per-local-user masked mean over items ----
            ps_sum = [
                psA.tile([1, n], f32, name=f"ps_sum{j}")
                for j, (o, n) in enumerate(nslices)
            ]
            ps_cnt = [
                psA.tile([1, n], f32, name=f"ps_cnt{j}")
                for j, (o, n) in enumerate(nslices)
            ]
            for g in range(ICH):
                rt = rpool.tile([128, W], bf16, name="rt")
                nc.sync.dma_start(out=rt[:], in_=rtt_t[g * 128 : (g + 1) * 128, :])
                mk = mpool.tile([128, UL], bf16, name="mk")
                # table holds (R - 2.5); a zero rating is the value -2.5
                nc.vector.tensor_scalar(
                    out=mk[:], in0=rt[:, :UL], scalar1=-2.5, scalar2=None,
                    op0=Alu.not_equal,
                )
                first, last = g == 0, g == ICH - 1
                for j, (o, n) in enumerate(nslices):
                    nc.tensor.matmul(
                        out=ps_sum[j][:], lhsT=ones_w[:], rhs=rt[:, o : o + n],
                        start=first, stop=last,
                    )
                    nc.tensor.matmul(
                        out=ps_cnt[j][:], lhsT=ones_w[:], rhs=mk[:, o : o + n],
                        start=first, stop=last,
                    )

            cnt_sb = st.tile([1, UL], f32)
            ubf_row = st.tile([1, W], f32)
            nc.gpsimd.memset(ubf_row[:], 0.0)
            for j, (o, n) in enumerate(nslices):
                nc.vector.tensor_scalar(
                    out=cnt_sb[:, o : o + n], in0=ps_cnt[j][:], scalar1=1.0,
                    scalar2=None, op0=Alu.max,
                )
            nc.vector.reciprocal(out=cnt_sb[:], in_=cnt_sb[:])
            # centering by 2.5: the A table holds (R - 2.5) (exact in bf16),
            # so ps_sum = sum(R) - 2.5*I. The weights become (ubf - 2.5) =
            # (ps_sum + 2.5*I)*recip - 2.5 -- the constant cancels in
            # G*(R-2.5) - G*(ubf-2.5). Shrinks |product| and with it the
            # bf16 product-rounding noise.
            for j, (o, n) in enumerate(nslices):
                nc.vector.tensor_scalar(
                    out=ubf_row[:, o : o + n], in0=ps_sum[j][:],
                    scalar1=2.5 * I, scalar2=None, op0=Alu.add,
                )
                nc.vector.tensor_tensor(
                    out=ubf_row[:, o : o + n], in0=ubf_row[:, o : o + n],
                    in1=cnt_sb[:, o : o + n], op=Alu.mult,
                )
            nc.vector.tensor_scalar(
                out=ubf_row[:, :UL], in0=ubf_row[:, :UL], scalar1=2.5,
                scalar2=None, op0=Alu.subtract,
            )
            # negate (we ADD the scalar inside scalar_tensor_tensor) and
            # scatter [1, W] -> [128, F] per-partition-scalar layout:
            # partition p, col f  <-  row element f*128+p
            nc.vector.tensor_scalar(
                out=ubf_row[:], in0=ubf_row[:], scalar1=-1.0, scalar2=None,
                op0=Alu.mult,
            )
            # bf16 is plenty for the centered ubf (|ubf-2.5| ~ 0.2, and the
            # scalar port must match the tensor dtype on HW). The [1, W] ->
            # [128, F] partition scatter runs as F K=1 matmuls (a strided
            # scatter DMA returns garbage on HW).
            ubf_row_d = st.tile([1, W], bf16)
            nc.vector.tensor_copy(out=ubf_row_d[:], in_=ubf_row[:])
            one1 = st.tile([1, 1], bf16)
            nc.gpsimd.memset(one1[:], 1.0)
            ps_ubf = psA.tile([128, F], f32, name="ps_ubf", tag="ps_cnt0")
            for f in range(F):
                nc.tensor.matmul(
                    out=ps_ubf[:, f : f + 1],
                    lhsT=ubf_row_d[:, f * 128 : (f + 1) * 128],
                    rhs=one1[:],
                    start=True,
                    stop=True,
                )
            ubf_colT = st.tile([128, F], bf16)
            nc.vector.tensor_copy(out=ubf_colT[:], in_=ps_ubf[:])

            # ---- main loop ----
            scores_row = st.tile([1, B], f32)
            icn = CH // 16
            for k in range(NCH):
                gk = gpool.tile([128, GF, CH], bf16, name="gk")
                ak = apool.tile([128, F, CH], bf16, name="ak")
                nc.gpsimd.dma_gather(
                    out_ap=gk[:], in_ap=sim_t[:],
                    idxs_ap=uidx_sb[:, k * icn : (k + 1) * icn],
                    num_idxs=CH, num_idxs_reg=CH, elem_size=WG,
                    transpose=True, queue_num=0,
                )
                nc.gpsimd.dma_gather(
                    out_ap=ak[:], in_ap=rtt_t[:],
                    idxs_ap=iidx_sb[:, k * icn : (k + 1) * icn],
                    num_idxs=CH, num_idxs_reg=CH, elem_size=W,
                    transpose=True, queue_num=1,
                )
                # raw products on DVE at 2x; the ubf term goes to the PE as
                # a second weighted reduce (ubf_colT holds -(ubf-2.5), so
                # the terms ADD): score = sum(G*A) + sum((-ubf_c)*G)
                p1 = ppool.tile([128, F, CH], bf16, name="p1")
                nc.vector.tensor_tensor(
                    out=p1[:], in0=gk[:, 0:F, :], in1=ak[:], op=Alu.mult
                )
                prods = [p1]
                if cfg.g_hilo:
                    p2 = ppool.tile([128, F, CH], bf16, name="p2")
                    nc.vector.tensor_tensor(
                        out=p2[:], in0=gk[:, F : 2 * F, :], in1=ak[:], op=Alu.mult
                    )
                    prods.append(p2)
                ps_p = psB.tile([1, CH], f32, name="ps_p")
                ps_u = psB.tile([1, CH], f32, name="ps_u")
                npr = len(prods) * F
                i = 0
                for src in prods:
                    for f in range(F):
                        nc.tensor.matmul(
                            out=ps_p[:], lhsT=ones_w[:], rhs=src[:, f, :],
                            start=(i == 0), stop=(i == npr - 1),
                        )
                        i += 1
                for ff in range(GF):
                    f = ff % F
                    nc.tensor.matmul(
                        out=ps_u[:], lhsT=ubf_colT[:, f : f + 1], rhs=gk[:, ff, :],
                        start=(ff == 0), stop=(ff == GF - 1),
                    )
                sc = scores_row[:, k * CH : (k + 1) * CH]
                # DVE (not ACT) copy: PE-W vs ACT-R same-bank isn't
                # serialized by the scheduler's bank tracker on HW
                nc.vector.tensor_copy(out=sc, in_=ps_p[:])
                nc.vector.tensor_tensor(
                    out=sc, in0=sc, in1=ps_u[:], op=Alu.add
                )

            # ---- finish: AllReduce (split in halves so the first one
            # overlaps the tail of the main loop) -> sigmoid -> out
            red_sb = st.tile([128, BC], f32)
            H = B // 2
            HP = 64  # partitions covered by one half in the [128, BC] view
            for h in range(2):
                pd = dram.tile([1, H], f32, name=f"part_d{h}")
                rd = dram.tile([1, H], f32, name=f"red_d{h}", addr_space="Shared")
                nc.sync.dma_start(
                    out=pd[:], in_=scores_row[:, h * H : (h + 1) * H]
                )
                nc.gpsimd.collective_compute(
                    "AllReduce", Alu.add, replica_groups=groups,
                    ins=[pd.opt()], outs=[rd.opt()],
                )
                nc.sync.dma_start(
                    out=red_sb[h * HP : (h + 1) * HP, :],
                    in_=rd[:].rearrange("o (p c) -> (o p) c", p=HP),
                )
            fin = st.tile([128, BC], f32)
            nc.scalar.activation(out=fin[:], in_=red_sb[:], func=Act.Sigmoid)
            nc.vector.tensor_scalar_mul(out=fin[:], in0=fin[:], scalar1=5.0)
            nc.sync.dma_start(
                out=out_t[:].rearrange("(p c) -> p c", p=128), in_=fin[:]
            )

    nc.compile()
    return nc


def build_program_b(cfg: Cfg):
    from concourse import bacc, mybir, tile
    from concourse.masks import make_identity

    f32 = mybir.dt.float32
    i16 = mybir.dt.int16
    dtd = mybir.dt.bfloat16 if cfg.use_bf16 else f32
    Alu = mybir.AluOpType
    Act = mybir.ActivationFunctionType

    U, I, B, W, UL = cfg.n_users, cfg.n_items, cfg.batch, cfg.w, cfg.ul
    CH = cfg.chunk
    SUB = CH // 128  # 128-batch subtiles per gather chunk
    T = B // 128  # score columns
    ICH = I // 128  # rating row-chunks for the ubf pass
    IDXC = B // 16  # index-table columns
    groups = [list(range(cfg.n_cores))]
    # ubf column slices (PSUM banks hold <=512 fp32 per partition)
    nslices = [(o, min(512, UL - o)) for o in range(0, UL, 512)]

    nc = bacc.Bacc(
        None, target_bir_lowering=False, debug=False, num_swdge_queues=2
    )

    sim_t = nc.dram_tensor("sim_aug", [U, W], dtd, kind="ExternalInput")
    rtt_t = nc.dram_tensor("ratt_aug", [I, W], dtd, kind="ExternalInput")
    uidx_t = nc.dram_tensor("uidx", [128, IDXC], i16, kind="ExternalInput")
    iidx_t = nc.dram_tensor("iidx", [128, IDXC], i16, kind="ExternalInput")
    out_t = nc.dram_tensor("out", [B], f32, kind="ExternalOutput")

    with tile.TileContext(nc) as tc:
        with (
            tc.tile_pool(name="static", bufs=1) as st,
            tc.tile_pool(name="rstream", bufs=4) as rpool,
            tc.tile_pool(name="maskp", bufs=2) as mpool,
            tc.tile_pool(name="gpool", bufs=4) as gpool,
            tc.tile_pool(name="apool", bufs=4) as apool,
            tc.tile_pool(name="dpool", bufs=6) as dpool,
            tc.tile_pool(name="psum", bufs=1, space="PSUM") as pp,
            tc.tile_pool(name="dram", bufs=1, space="DRAM") as dram,
        ):
            # ---- static setup ----
            ones = st.tile([128, 1], dtd)
            nc.gpsimd.memset(ones[:], 1.0)
            ident = st.tile([128, 128], f32)
            make_identity(nc, ident[:])
            uidx_sb = st.tile([128, IDXC], i16)
            nc.sync.dma_start(out=uidx_sb[:], in_=uidx_t[:])
            iidx_sb = st.tile([128, IDXC], i16)
            nc.sync.dma_start(out=iidx_sb[:], in_=iidx_t[:])

            # ---- ubf: per-local-user masked mean over items ----
            ps_sum = [
                pp.tile([1, n], f32, name=f"ps_sum{j}")
                for j, (o, n) in enumerate(nslices)
            ]
            ps_cnt = [
                pp.tile([1, n], f32, name=f"ps_cnt{j}")
                for j, (o, n) in enumerate(nslices)
            ]
            for g in range(ICH):
                rt = rpool.tile([128, W], dtd, name="rt")
                nc.sync.dma_start(out=rt[:], in_=rtt_t[g * 128 : (g + 1) * 128, :])
                mk = mpool.tile([128, UL], dtd, name="mk")
                nc.vector.tensor_scalar(
                    out=mk[:],
                    in0=rt[:, :UL],
                    scalar1=0.0,
                    scalar2=None,
                    op0=Alu.not_equal,
                )
                first, last = g == 0, g == ICH - 1
                for j, (o, n) in enumerate(nslices):
                    nc.tensor.matmul(
                        out=ps_sum[j][:],
                        lhsT=ones[:],
                        rhs=rt[:, o : o + n],
                        start=first,
                        stop=last,
                    )
                    nc.tensor.matmul(
                        out=ps_cnt[j][:],
                        lhsT=ones[:],
                        rhs=mk[:, o : o + n],
                        start=first,
                        stop=last,
                    )

            cnt_sb = st.tile([1, UL], f32)
            ubf_row = st.tile([1, W], f32)
            nc.gpsimd.memset(ubf_row[:], 0.0)
            for j, (o, n) in enumerate(nslices):
                nc.vector.tensor_scalar(
                    out=cnt_sb[:, o : o + n],
                    in0=ps_cnt[j][:],
                    scalar1=1.0,
                    scalar2=None,
                    op0=Alu.max,
                )
            nc.vector.reciprocal(out=cnt_sb[:], in_=cnt_sb[:])
            for j, (o, n) in enumerate(nslices):
                nc.vector.tensor_tensor(
                    out=ubf_row[:, o : o + n],
                    in0=ps_sum[j][:],
                    in1=cnt_sb[:, o : o + n],
                    op=Alu.mult,
                )
            ubf_row_d = st.tile([1, W], dtd)
            nc.vector.tensor_copy(out=ubf_row_d[:], in_=ubf_row[:])
            ubf_bcast = st.tile([128, W], dtd)
            nc.gpsimd.partition_broadcast(out_ap=ubf_bcast[:], in_ap=ubf_row_d[:])

            # ---- main loop: gather G/A rows; (A-ubf)*G on DVE, reduce on ACT
            scores = st.tile([128, T], f32)
            icn = CH // 16
            for k in range(B // CH):
                gk = gpool.tile([128, SUB, W], dtd, name="gk")
                ak = apool.tile([128, SUB, W], dtd, name="ak")
                nc.gpsimd.dma_gather(
                    out_ap=gk[:],
                    in_ap=sim_t[:],
                    idxs_ap=uidx_sb[:, k * icn : (k + 1) * icn],
                    num_idxs=CH,
                    num_idxs_reg=CH,
                    elem_size=W,
                    queue_num=0,
                )
                nc.gpsimd.dma_gather(
                    out_ap=ak[:],
                    in_ap=rtt_t[:],
                    idxs_ap=iidx_sb[:, k * icn : (k + 1) * icn],
                    num_idxs=CH,
                    num_idxs_reg=CH,
                    elem_size=W,
                    queue_num=1,
                )
                for c in range(SUB):
                    d = dpool.tile([128, W], dtd, name="d")
                    nc.vector.tensor_tensor(
                        out=d[:], in0=ak[:, c, :], in1=ubf_bcast[:], op=Alu.subtract
                    )
                    nc.vector.tensor_tensor(
                        out=d[:], in0=d[:], in1=gk[:, c, :], op=Alu.mult
                    )
                    t_i = k * SUB + c
                    nc.scalar.activation(
                        out=d[:],
                        in_=d[:],
                        func=Act.Copy,
                        accum_out=scores[:, t_i : t_i + 1],
                    )

            # ---- finish: transpose -> AllReduce -> sigmoid -> out ----
            ps_t = pp.tile([T, 128], f32, name="ps_t")
            nc.tensor.transpose(out=ps_t[:], in_=scores[:], identity=ident[:])
            sc_t = st.tile([T, 128], f32)
            nc.scalar.copy(out=sc_t[:], in_=ps_t[:])
            part_d = dram.tile([T, 128], f32, name="part_d")
            red_d = dram.tile([T, 128], f32, name="red_d", addr_space="Shared")
            nc.sync.dma_start(out=part_d[:], in_=sc_t[:])
            nc.gpsimd.collective_compute(
                "AllReduce",
                Alu.add,
                replica_groups=groups,
                ins=[part_d.opt()],
                outs=[red_d.opt()],
            )
            red_sb = st.tile([T, 128], f32)
            nc.sync.dma_start(out=red_sb[:], in_=red_d[:])
            fin = st.tile([T, 128], f32)
            nc.scalar.activation(out=fin[:], in_=red_sb[:], func=Act.Sigmoid)
            nc.vector.tensor_scalar_mul(out=fin[:], in0=fin[:], scalar1=5.0)
            nc.sync.dma_start(
                out=out_t[:].rearrange("(t p) -> t p", p=128), in_=fin[:]
            )

    nc.compile()
    return nc


def make_in_maps(cfg, user, item, rating_mtx, user_similarity, user_bias, item_bias, global_bias):
    import ml_dtypes

    U, I, B, W, UL = cfg.n_users, cfg.n_items, cfg.batch, cfg.w, cfg.ul
    npd = ml_dtypes.bfloat16 if cfg.use_bf16 else np.float32
    u_i = np.asarray(user).astype(np.int64)
    i_i = np.asarray(item).astype(np.int64)
    sim = np.asarray(user_similarity, dtype=np.float32)
    R = np.asarray(rating_mtx, dtype=np.float32)
    ub = np.asarray(user_bias, dtype=np.float32)
    ibg = np.asarray(item_bias, dtype=np.float32) + np.float32(
        np.asarray(global_bias)
    )

    def hilo(x):
        hi = x.astype(npd)
        lo = (x - hi.astype(np.float32)).astype(npd)
        return hi, lo

    ub_hi, ub_lo = hilo(ub)
    ib_hi, ib_lo = hilo(ibg)

    # idx layout: [16, B/16] block (idx i at [i%16, i//16]) tiled 8x down the
    # partition axis -- each GPSIMD Q7 core reads its own 16-partition replica
    uidx = np.tile(u_i.astype(np.int16).reshape(B // 16, 16).T, (8, 1))
    iidx = np.tile(i_i.astype(np.int16).reshape(B // 16, 16).T, (8, 1))

    hilo = cfg.layout_t and cfg.g_hilo
    maps = []
    for k in range(cfg.n_cores):
        s_slice = sim[:, k * UL : (k + 1) * UL]
        s_hi = s_slice.astype(npd)
        wg = 2 * W if hilo else W
        sa = np.zeros((U, wg), npd)
        sa[:, :UL] = s_hi
        ra = np.zeros((I, W), npd)
        r_shift = 2.5 if cfg.layout_t else 0.0
        ra[:, :UL] = (R[k * UL : (k + 1) * UL, :].T - r_shift).astype(npd)
        if hilo:
            sa[:, W : W + UL] = (s_slice - s_hi.astype(np.float32)).astype(npd)
        if k == 0:
            ra[:, UL] = 1.0
            ra[:, UL + 1] = 1.0
            ra[:, UL + 2] = ib_hi
            ra[:, UL + 3] = ib_lo
            if hilo:
                sa[:, UL] = ub_hi
                sa[:, W + UL] = ub_lo
                sa[:, UL + 2] = 1.0
                sa[:, UL + 3] = 1.0
            else:
                sa[:, UL] = ub_hi
                sa[:, UL + 1] = ub_lo
                sa[:, UL + 2] = 1.0
                sa[:, UL + 3] = 1.0
        maps.append(
            {"sim_aug": sa, "ratt_aug": ra, "uidx": uidx, "iidx": iidx}
        )
    return maps


_PROGRAM_CACHE = {}


def _get_program(cfg: Cfg):
    if cfg not in _PROGRAM_CACHE:
        _PROGRAM_CACHE[cfg] = build_program(cfg)
    return _PROGRAM_CACHE[cfg]


def kernel(user, item, rating_mtx, user_similarity, user_bias, item_bias, global_bias):
    from concourse import bass_utils

    cfg = Cfg()
    assert np.asarray(rating_mtx).shape == (cfg.n_users, cfg.n_items)
    assert np.asarray(user).shape == (cfg.batch,)
    nc = _get_program(cfg)
    in_maps = make_in_maps(
        cfg, user, item, rating_mtx, user_similarity, user_bias, item_bias, global_bias
    )
    res = bass_utils.run_bass_kernel_spmd(
        nc, in_maps, core_ids=list(range(cfg.n_cores))
    )
    return np.asarray(res.results[0]["out"], dtype=np.float32).reshape(cfg.batch)

